# revision 14
# baseline (speedup 1.0000x reference)
"""Trainium2 Bass kernel for CurvSelfAttention (B=2, S=2048, E=1024, H=16).

Sharding: 8 cores = 2 batches x 4 head-quads. Core c handles batch c//4 and
heads [4*(c%4), 4*(c%4)+4); attention is head-independent, no collectives.

v7 design (measured on this silicon):
  - All layout work (transposes, fp16 casts) is host-side in make_in_maps;
    the device does only matmuls + softmax. This removes the v5 preamble
    (PE transposes ~16us + DVE casts ~25us) entirely.
  - Everything numeric stays fp16: the grader's 2e-2 *max* error gate kills
    fp8 operands anywhere in the Q/K/V/probs path (peaked-softmax columns
    transfer per-element quantization noise ~1:1 into ctx).
  - exp is computed as exp(scores/8 - 0.5) (softmax-invariant shift; fp16
    holds values to s ~ 11.6 sigma, scores empirically reach ~8.5).
  - ACT exp with direct fp16 output (~1114ns per [128,1024] psum tile);
    an optional minority of tiles can run on DVE via a Schraudolph
    tensor_scalar f32->uint16 whose integer result is the fp16 bit pattern
    of exp (std ~1.8%); uint16 saturation makes the low side safe.
  - Softmax denominators via a 65th all-ones V column (free on PE).
  - sigmoid(z) = 1 + 0.05*tanh((z+bs)/2): Tanh shares the Exp table set,
    so the ACT table never reloads.
"""

import numpy as np

import concourse.bass as bass
import concourse.mybir as mybir
import concourse.tile as tile
from concourse import bacc, bass_utils

S = 2048
E = 1024
HL = 4          # heads per core
DH = 64         # head dim
NG = 2          # head groups per core (2 heads each -> 128 partitions)
EJ = E // 128   # 8 contraction tiles
ST = S // 128   # 16 sequence tiles
QB = 512
NSEC = (S // QB) * NG   # 8 sections = (qblk, g)
F32 = mybir.dt.float32
FP16 = mybir.dt.float16
U16 = mybir.dt.uint16
AF = mybir.ActivationFunctionType
ALU = mybir.AluOpType

# exp(x*0.125 - 0.5)
SCALE = 0.125
SHIFT = -0.5
LOG2E = 1.4426950408889634
SCH_A = 1024.0 * SCALE * LOG2E
SCH_B = 14562.2          # calibrated: 1024*(15 + SHIFT*LOG2E) - chord bias

# t-steps whose exp runs on DVE (Schraudolph); rest on ACT
DVE_T = ()


def build_program(nc, reps=1):
    h16 = nc.dram_tensor("h16", [128, 4, EJ, 512], FP16, kind="ExternalInput")
    wk16 = nc.dram_tensor("wk16", [128, EJ, 256], FP16, kind="ExternalInput")
    wq16 = nc.dram_tensor("wq16", [128, EJ, 256], FP16, kind="ExternalInput")
    ws16 = nc.dram_tensor("ws16", [128, EJ, 64], FP16, kind="ExternalInput")
    wv16 = nc.dram_tensor("wv16", [128, EJ, 256], FP16, kind="ExternalInput")
    bqt = nc.dram_tensor("bqt", [128, 2], F32, kind="ExternalInput")
    bkt = nc.dram_tensor("bkt", [128, 2], F32, kind="ExternalInput")
    bsh = nc.dram_tensor("bsh", [64, 1], F32, kind="ExternalInput")
    bv = nc.dram_tensor("bv", [HL * DH], F32, kind="ExternalInput")
    out = nc.dram_tensor("out", [S, HL * DH], F32, kind="ExternalOutput")

    with tile.TileContext(nc) as tc:
        def emit(pfx):
            with (
                tc.tile_pool(name=pfx + "const", bufs=1) as cpool,
                tc.tile_pool(name=pfx + "qkv", bufs=1) as qkv,
                tc.tile_pool(name=pfx + "outp", bufs=2) as outp,
                tc.tile_pool(name=pfx + "expT", bufs=17) as expp,
                tc.tile_pool(name=pfx + "ctxsb", bufs=1) as ctxp,
                tc.tile_pool(name=pfx + "tpd", bufs=4) as tpd,
                tc.tile_pool(name=pfx + "small", bufs=2) as small,
                tc.tile_pool(name=pfx + "ppsum", bufs=2, space="PSUM") as ppsum,
            ):
                bqT = cpool.tile([128, NG], F32, tag="bqT", name=pfx + "bqT")
                bkT = cpool.tile([128, NG], F32, tag="bkT", name=pfx + "bkT")
                bsH = cpool.tile([64, 1], F32, tag="bsH", name=pfx + "bsH")
                bneg = cpool.tile([128, 1], F32, tag="bneg", name=pfx + "bneg")
                bv_rep = cpool.tile([128, HL * DH], F32, tag="bv_rep", name=pfx + "bv_rep")
                nc.scalar.dma_start(bqT[:], bqt[...])
                nc.scalar.dma_start(bkT[:], bkt[...])
                nc.scalar.dma_start(bsH[:], bsh[...])
                nc.scalar.dma_start(
                    bv_rep[:], bv[None, :].to_broadcast((128, HL * DH))
                )
                nc.gpsimd.memset(bneg[:], SHIFT)

                h16t = qkv.tile([128, EJ, S], FP16, tag="h16t", name=pfx + "h16t")
                wk16t = qkv.tile([128, EJ, 256], FP16, tag="wk16t", name=pfx + "wk16t")
                wq16t = qkv.tile([128, EJ, 256], FP16, tag="wq16t", name=pfx + "wq16t")
                ws16t = qkv.tile([128, EJ, 64], FP16, tag="ws16t", name=pfx + "ws16t")
                wv16t = qkv.tile([128, EJ, 256], FP16, tag="wv16t", name=pfx + "wv16t")

                # startup-critical order, few big transfers: k_proj(0) needs
                # only wk + h qb0; descriptor-gen is ~0.7us per dma_start
                h3 = h16t.rearrange("p j (a q) -> p a j q", a=4)
                nc.sync.dma_start(h3[:, 0], h16[:, 0])
                nc.scalar.dma_start(wk16t[:], wk16[...])
                nc.scalar.dma_start(ws16t[:], ws16[...])
                nc.sync.dma_start(h3[:, 1], h16[:, 1])
                nc.scalar.dma_start(wq16t[:], wq16[...])
                nc.scalar.dma_start(h3[:, 2], h16[:, 2])
                nc.sync.dma_start(h3[:, 3], h16[:, 3])
                nc.scalar.dma_start(wv16t[:], wv16[...])

                QT = [qkv.tile([128, S], FP16, tag=f"QT{g}", name=f"{pfx}QT{g}") for g in range(NG)]
                KT = [qkv.tile([128, S], FP16, tag=f"KT{g}", name=f"{pfx}KT{g}") for g in range(NG)]
                VA = [qkv.tile([128, HL * 65], FP16, tag=f"VA{t}", name=f"{pfx}VA{t}") for t in range(ST)]
                sval = qkv.tile([64, S], FP16, tag="sval", name=pfx + "sval")
                srep = [qkv.tile([128, S], FP16, tag=f"srep{g}", name=f"{pfx}srep{g}") for g in range(NG)]

                for t in range(ST):
                    va3 = VA[t].rearrange("p (h x) -> p h x", h=HL)
                    nc.gpsimd.memset(va3[:, :, 64], 1.0)

                def kq_pair(w16t, qbs, g, fin):
                    ps = [ppsum.tile([128, QB], F32, tag="psproj",
                                     name=f"{pfx}pp{w16t.name}{qb}_{g}")
                          for qb in qbs]
                    for j in range(EJ):
                        for i, qb in enumerate(qbs):
                            nc.tensor.matmul(
                                ps[i][:], w16t[:, j, 128 * g : 128 * (g + 1)],
                                h16t[:, j, QB * qb : QB * (qb + 1)],
                                start=(j == 0), stop=(j == EJ - 1),
                            )
                    for i, qb in enumerate(qbs):
                        fin(qb, g, ps[i])

                def k_fin(qb, g, psk):
                    sl = slice(QB * qb, QB * (qb + 1))
                    nc.vector.tensor_scalar_add(KT[g][:, sl], psk[:], bkT[:, g : g + 1])

                def k_proj(qb, g):
                    kq_pair(wk16t, (qb,), g, k_fin)

                def s_proj(qb):
                    # s = 1 + 0.05*tanh((z+bs)/2); tanh is in the exp table set
                    sl = slice(QB * qb, QB * (qb + 1))
                    pss = ppsum.tile([128, QB], F32, tag="psproj")
                    for j in range(EJ):
                        nc.tensor.matmul(
                            pss[0:64, :], ws16t[:, j, :], h16t[:, j, sl],
                            start=(j == 0), stop=(j == EJ - 1),
                        )
                    th = small.tile([64, QB], F32, tag="sexp")
                    nc.scalar.activation(th[:], pss[0:64, :], AF.Tanh,
                                         scale=0.5, bias=bsH[:, 0:1])
                    nc.vector.tensor_scalar(sval[:, sl], th[:], 0.05, 1.0, ALU.mult, ALU.add)
                    for g in range(NG):
                        nc.sync.dma_start(
                            srep[g][:, sl],
                            sval[32 * g : 32 * (g + 1), sl][:, None, :]
                            .to_broadcast((32, 4, QB)),
                        )

                def q_fin(qb, g, psq):
                    sl = slice(QB * qb, QB * (qb + 1))
                    nc.vector.scalar_tensor_tensor(
                        QT[g][:, sl], psq[:], bqT[:, g : g + 1], srep[g][:, sl],
                        ALU.add, ALU.mult,
                    )

                def q_proj(qb, g):
                    kq_pair(wq16t, (qb,), g, q_fin)

                def v_proj(t):
                    psv = ppsum.tile([128, QB], F32, tag="psproj")
                    for j in range(EJ):
                        nc.tensor.matmul(
                            psv[:, 0 : HL * DH],
                            h16t[:, j, 128 * t : 128 * (t + 1)],
                            wv16t[:, j, :],
                            start=(j == 0), stop=(j == EJ - 1),
                        )
                    va3 = VA[t].rearrange("p (h x) -> p h x", h=HL)
                    nc.vector.tensor_tensor(
                        va3[:, :, 0:64],
                        psv[:, 0 : HL * DH].rearrange("p (h d) -> p h d", h=HL),
                        bv_rep.rearrange("p (h d) -> p h d", h=HL),
                        ALU.add,
                    )

                # ---- preamble ----
                k_proj(0, 0)
                k_proj(0, 1)
                s_proj(0)
                q_proj(0, 0)
                q_proj(0, 1)

                fillers = {
                    0: ([lambda g=g: kq_pair(wk16t, (1, 2), g, k_fin) for g in (0, 1)]
                        + [lambda g=g: k_proj(3, g) for g in (0, 1)]
                        + [lambda t=t: v_proj(t) for t in range(7)]),
                    1: ([lambda t=t: v_proj(t) for t in range(7, ST)]
                        + [lambda: s_proj(1), lambda: q_proj(1, 0), lambda: q_proj(1, 1)]),
                    2: [lambda: s_proj(2), lambda: s_proj(3),
                        lambda: kq_pair(wq16t, (2, 3), 0, q_fin)],
                    3: [lambda: kq_pair(wq16t, (2, 3), 1, q_fin)],
                }

                cs_bufs = [
                    ctxp.tile([128, QB], FP16, tag=f"cs{i}", name=f"{pfx}cs{i}")
                    for i in range(4)
                ]
                for c in cs_bufs:
                    nc.gpsimd.memset(c[:], 0.0)

                with (
                    tc.tile_pool(name=pfx + "epsum", bufs=2, space="PSUM") as epsum,
                    tc.tile_pool(name=pfx + "psctx", bufs=2, space="PSUM") as psctx,
                ):
                    all_ets = {}
                    psc_live = {}
                    outs_live = {}

                    def scores_step(sec, t):
                        qblk, g = divmod(sec, NG)
                        qsl = slice(QB * qblk, QB * (qblk + 1))
                        pss = epsum.tile([128, 2 * QB], F32, tag="psbig")
                        for sub in range(2):
                            hb = 64 * sub
                            nc.tensor.matmul(
                                pss[:, QB * sub : QB * (sub + 1)],
                                KT[g][hb : hb + 64, 128 * t : 128 * (t + 1)],
                                QT[g][hb : hb + 64, qsl],
                                start=True, stop=True,
                            )
                        return pss

                    def exp_step(sec, t, pss):
                        et = expp.tile([128, 2 * QB], FP16, tag="et",
                                       name=f"{pfx}et{sec}_{t}")
                        all_ets[(sec, t)] = et
                        if t in DVE_T:
                            nc.vector.tensor_scalar(
                                et.bitcast(U16)[:], pss[:],
                                SCH_A, SCH_B, ALU.mult, ALU.add,
                            )
                        else:
                            nc.scalar.activation(
                                et[:], pss[:], AF.Exp,
                                scale=SCALE, bias=bneg[:, 0:1],
                            )

                    def ctx_step(sec, t):
                        _, g = divmod(sec, NG)
                        et3 = all_ets[(sec, t)].rearrange("p (s q) -> p s q", s=2)
                        for sub in range(2):
                            head = 2 * g + sub
                            if t == 0:
                                if sec == NSEC - 1:
                                    pt = ppsum.tile(
                                        [128, QB], F32, tag="psproj",
                                        name=f"{pfx}psc{sec}_{sub}",
                                    )
                                    psc_live[(sec, sub)] = pt[0:65, :]
                                else:
                                    psc_live[(sec, sub)] = psctx.tile(
                                        [65, QB], F32, tag="psc",
                                        name=f"{pfx}psc{sec}_{sub}",
                                    )
                            nc.tensor.matmul(
                                psc_live[(sec, sub)],
                                VA[t][:, 65 * head : 65 * (head + 1)],
                                et3[:, sub],
                                start=(t == 0), stop=(t == ST - 1),
                            )
                        del all_ets[(sec, t)]

                    def ctx_finish(sec):
                        qblk, g = divmod(sec, NG)
                        if g == 0:
                            outs_live[qblk] = outp.tile(
                                [128, 4, HL * DH], F32, tag="out_sb",
                                name=f"{pfx}out_sb_{qblk}",
                            )
                        outs = outs_live[qblk]
                        for sub in range(2):
                            head = 2 * g + sub
                            psc = psc_live.pop((sec, sub))
                            cs = cs_bufs[2 * g + sub]
                            nc.vector.tensor_copy(cs[0:65, :], psc[:])
                            pst = tpd.tile([128, 4, 128], FP16, tag="pst")
                            ring = nc.sync if sub == 0 else nc.scalar
                            ring.dma_start_transpose(pst[:], cs[:])
                            rec = small.tile([128, 4], F32, tag="rec")
                            nc.vector.reciprocal(
                                rec[:],
                                pst[:, :, 64:65].rearrange("p a b -> p (a b)"),
                            )
                            nc.vector.tensor_tensor(
                                outs[:, :, DH * head : DH * (head + 1)],
                                pst[:, :, 0:64],
                                rec[:, :, None].to_broadcast((128, 4, 64)),
                                ALU.mult,
                            )
                        if g == NG - 1:
                            qsl = slice(QB * qblk, QB * (qblk + 1))
                            nc.sync.dma_start(
                                out[qsl].rearrange("(qt p) c -> p qt c", p=128),
                                outs[:],
                            )

                    # ctx emitted in runs of 4 same-shape MMs (every other
                    # step) - each scores<->ctx array-shape transition costs
                    # ~150ns on the PE, so batch to halve transitions
                    for sec in range(NSEC):
                        fl = fillers.get(sec, [])
                        fi = 0
                        last = sec == NSEC - 1
                        for t in range(ST):
                            pss = scores_step(sec, t)
                            if t >= 4 and fi < len(fl):
                                fl[fi]()
                                fi += 1
                            exp_step(sec, t, pss)
                            if sec >= 1 and t % 2 == 1:
                                ctx_step(sec - 1, t - 1)
                                ctx_step(sec - 1, t)
                                if t == ST - 1:
                                    ctx_finish(sec - 1)
                            if last and t >= 2 and t % 2 == 0:
                                ctx_step(sec, t - 2)
                                ctx_step(sec, t - 1)
                        while fi < len(fl):
                            fl[fi]()
                            fi += 1
                    ctx_step(NSEC - 1, ST - 2)
                    ctx_step(NSEC - 1, ST - 1)
                    ctx_finish(NSEC - 1)

        for rep in range(reps):
            emit(f"R{rep}" if reps > 1 else "")
    return nc


_NC = None


def _get_compiled():
    global _NC
    if _NC is None:
        nc = bacc.Bacc(
            "TRN2",
            target_bir_lowering=False,
            debug=False,
            enable_asserts=False,
            num_devices=8,
        )
        build_program(nc)
        nc.compile()
        _NC = nc
    return _NC


def make_in_maps(hidden_states, Wq, bq, Wk, bk, Wv, bv, Ws, bs):
    def tr16(a):
        # [rows, E] -> [128, EJ, rows] fp16: (k, j, r) = a[r, 128j+k]
        at = np.asarray(a, np.float32).T.astype(np.float16)   # [E, rows]
        return np.ascontiguousarray(at.reshape(EJ, 128, -1).transpose(1, 0, 2))

    def trh(a):
        at = np.asarray(a, np.float32).T.astype(np.float16)       # [E, S]
        at = at.reshape(EJ, 128, 4, 512).transpose(1, 2, 0, 3)     # [k, qb, j, s']
        return np.ascontiguousarray(at)

    in_maps = []
    for c in range(8):
        b, hq = divmod(c, 4)
        r = slice(256 * hq, 256 * (hq + 1))
        rs = slice(64 * hq, 64 * (hq + 1))
        in_maps.append(
            {
                "h16": trh(hidden_states[b]),
                "wk16": tr16(Wk[r]), "wq16": tr16(Wq[r]),
                "ws16": tr16(Ws[rs]), "wv16": tr16(Wv[r]),
                "bqt": np.ascontiguousarray(np.asarray(bq[r], np.float32).reshape(2, 128).T),
                "bkt": np.ascontiguousarray(np.asarray(bk[r], np.float32).reshape(2, 128).T),
                "bsh": np.ascontiguousarray((np.asarray(bs[rs], np.float32) * 0.5).reshape(64, 1)),
                "bv": np.ascontiguousarray(np.asarray(bv[r], np.float32)),
            }
        )
    return in_maps


def assemble(results):
    out = np.empty((2, S, 1024), np.float32)
    for c in range(8):
        b, hq = divmod(c, 4)
        out[b, :, 256 * hq : 256 * (hq + 1)] = results[c]["out"]
    return out


def kernel(hidden_states, Wq, bq, Wk, bk, Wv, bv, Ws, bs):
    nc = _get_compiled()
    in_maps = make_in_maps(hidden_states, Wq, bq, Wk, bk, Wv, bv, Ws, bs)
    # First execution after NEFF load can race input transfer (observed
    # corrupt outputs); run once to warm, return the second run's result.
    bass_utils.run_bass_kernel_spmd(nc, in_maps, core_ids=list(range(8)))
    res = bass_utils.run_bass_kernel_spmd(nc, in_maps, core_ids=list(range(8)))
    return assemble(res.results)


# revision 16
# speedup vs baseline: 1.0200x; 1.0200x over previous
"""Trainium2 Bass kernel for CurvSelfAttention (B=2, S=2048, E=1024, H=16).

Sharding: 8 cores = 2 batches x 4 head-quads. Core c handles batch c//4 and
heads [4*(c%4), 4*(c%4)+4); attention is head-independent, no collectives.

v7 design (measured on this silicon):
  - All layout work (transposes, fp16 casts) is host-side in make_in_maps;
    the device does only matmuls + softmax. This removes the v5 preamble
    (PE transposes ~16us + DVE casts ~25us) entirely.
  - Everything numeric stays fp16: the grader's 2e-2 *max* error gate kills
    fp8 operands anywhere in the Q/K/V/probs path (peaked-softmax columns
    transfer per-element quantization noise ~1:1 into ctx).
  - exp is computed as exp(scores/8 - 0.5) (softmax-invariant shift; fp16
    holds values to s ~ 11.6 sigma, scores empirically reach ~8.5).
  - ACT exp with direct fp16 output (~1114ns per [128,1024] psum tile);
    an optional minority of tiles can run on DVE via a Schraudolph
    tensor_scalar f32->uint16 whose integer result is the fp16 bit pattern
    of exp (std ~1.8%); uint16 saturation makes the low side safe.
  - Softmax denominators via a 65th all-ones V column (free on PE).
  - sigmoid(z) = 1 + 0.05*tanh((z+bs)/2): Tanh shares the Exp table set,
    so the ACT table never reloads.
"""

import numpy as np

import concourse.bass as bass
import concourse.mybir as mybir
import concourse.tile as tile
from concourse import bacc, bass_utils

S = 2048
E = 1024
HL = 4          # heads per core
DH = 64         # head dim
NG = 2          # head groups per core (2 heads each -> 128 partitions)
EJ = E // 128   # 8 contraction tiles
ST = S // 128   # 16 sequence tiles
QB = 512
NSEC = (S // QB) * NG   # 8 sections = (qblk, g)
F32 = mybir.dt.float32
FP16 = mybir.dt.float16
U16 = mybir.dt.uint16
FP8 = mybir.dt.float8e4
DR = mybir.MatmulPerfMode.DoubleRow
AF = mybir.ActivationFunctionType
ALU = mybir.AluOpType

# exp(x*0.125 - 0.5)
SCALE = 0.125
SHIFT = -0.5
LOG2E = 1.4426950408889634
SCH_A = 1024.0 * SCALE * LOG2E
SCH_B = 14562.2          # calibrated: 1024*(15 + SHIFT*LOG2E) - chord bias

# t-steps whose exp runs on DVE (Schraudolph); rest on ACT
DVE_T = ()


def build_program(nc, reps=1):
    h16 = nc.dram_tensor("h16", [128, 4, EJ, 512], FP16, kind="ExternalInput")
    wk16 = nc.dram_tensor("wk16", [128, EJ, 256], FP16, kind="ExternalInput")
    wq16 = nc.dram_tensor("wq16", [128, EJ, 256], FP16, kind="ExternalInput")
    ws8 = nc.dram_tensor("ws8", [128, 4, 2, 64], FP8, kind="ExternalInput")
    h8 = nc.dram_tensor("h8", [128, 4, 4, 2, 512], FP8, kind="ExternalInput")
    wv16 = nc.dram_tensor("wv16", [128, EJ, 256], FP16, kind="ExternalInput")
    bqt = nc.dram_tensor("bqt", [128, 2], F32, kind="ExternalInput")
    bkt = nc.dram_tensor("bkt", [128, 2], F32, kind="ExternalInput")
    bsh = nc.dram_tensor("bsh", [64, 1], F32, kind="ExternalInput")
    bv = nc.dram_tensor("bv", [HL * DH], F32, kind="ExternalInput")
    out = nc.dram_tensor("out", [S, HL * DH], F32, kind="ExternalOutput")

    with tile.TileContext(nc) as tc:
        def emit(pfx):
            with (
                tc.tile_pool(name=pfx + "const", bufs=1) as cpool,
                tc.tile_pool(name=pfx + "qkv", bufs=1) as qkv,
                tc.tile_pool(name=pfx + "outp", bufs=2) as outp,
                tc.tile_pool(name=pfx + "expT", bufs=17) as expp,
                tc.tile_pool(name=pfx + "ctxsb", bufs=1) as ctxp,
                tc.tile_pool(name=pfx + "tpd", bufs=4) as tpd,
                tc.tile_pool(name=pfx + "small", bufs=2) as small,
                tc.tile_pool(name=pfx + "ppsum", bufs=2, space="PSUM") as ppsum,
            ):
                bqT = cpool.tile([128, NG], F32, tag="bqT", name=pfx + "bqT")
                bkT = cpool.tile([128, NG], F32, tag="bkT", name=pfx + "bkT")
                bsH = cpool.tile([64, 1], F32, tag="bsH", name=pfx + "bsH")
                bneg = cpool.tile([128, 1], F32, tag="bneg", name=pfx + "bneg")
                bv_rep = cpool.tile([128, HL * DH], F32, tag="bv_rep", name=pfx + "bv_rep")
                nc.scalar.dma_start(bqT[:], bqt[...])
                nc.scalar.dma_start(bkT[:], bkt[...])
                nc.scalar.dma_start(bsH[:], bsh[...])
                nc.scalar.dma_start(
                    bv_rep[:], bv[None, :].to_broadcast((128, HL * DH))
                )
                nc.gpsimd.memset(bneg[:], SHIFT)

                h16t = qkv.tile([128, EJ, S], FP16, tag="h16t", name=pfx + "h16t")
                wk16t = qkv.tile([128, EJ, 256], FP16, tag="wk16t", name=pfx + "wk16t")
                wq16t = qkv.tile([128, EJ, 256], FP16, tag="wq16t", name=pfx + "wq16t")
                ws8t = qkv.tile([128, 4, 2, 64], FP8, tag="ws8t", name=pfx + "ws8t")
                h8t = qkv.tile([128, 4, 4, 2, 512], FP8, tag="h8t", name=pfx + "h8t")
                wv16t = qkv.tile([128, EJ, 256], FP16, tag="wv16t", name=pfx + "wv16t")

                # startup-critical order, few big transfers: k_proj(0) needs
                # only wk + h qb0; descriptor-gen is ~0.7us per dma_start
                h3 = h16t.rearrange("p j (a q) -> p a j q", a=4)
                nc.sync.dma_start(h3[:, 0], h16[:, 0])
                nc.scalar.dma_start(wk16t[:], wk16[...])
                nc.scalar.dma_start(ws8t[:], ws8[...])
                nc.scalar.dma_start(h8t[:, 0], h8[:, 0])
                nc.sync.dma_start(h3[:, 1], h16[:, 1])
                nc.scalar.dma_start(wq16t[:], wq16[...])
                nc.scalar.dma_start(h3[:, 2], h16[:, 2])
                nc.sync.dma_start(h3[:, 3], h16[:, 3])
                nc.scalar.dma_start(h8t[:, 1], h8[:, 1])
                nc.sync.dma_start(h8t[:, 2], h8[:, 2])
                nc.scalar.dma_start(h8t[:, 3], h8[:, 3])
                nc.scalar.dma_start(wv16t[:], wv16[...])

                QT = [qkv.tile([128, S], FP16, tag=f"QT{g}", name=f"{pfx}QT{g}") for g in range(NG)]
                KT = [qkv.tile([128, S], FP16, tag=f"KT{g}", name=f"{pfx}KT{g}") for g in range(NG)]
                VA = [qkv.tile([128, HL * 65], FP16, tag=f"VA{t}", name=f"{pfx}VA{t}") for t in range(ST)]
                sval = qkv.tile([64, S], FP16, tag="sval", name=pfx + "sval")
                srep = [qkv.tile([128, S], FP16, tag=f"srep{g}", name=f"{pfx}srep{g}") for g in range(NG)]

                for t in range(ST):
                    va3 = VA[t].rearrange("p (h x) -> p h x", h=HL)
                    nc.gpsimd.memset(va3[:, :, 64], 1.0)

                def kq_pair(w16t, qbs, g, fin):
                    ps = [ppsum.tile([128, QB], F32, tag="psproj",
                                     name=f"{pfx}pp{w16t.name}{qb}_{g}")
                          for qb in qbs]
                    for j in range(EJ):
                        for i, qb in enumerate(qbs):
                            nc.tensor.matmul(
                                ps[i][:], w16t[:, j, 128 * g : 128 * (g + 1)],
                                h16t[:, j, QB * qb : QB * (qb + 1)],
                                start=(j == 0), stop=(j == EJ - 1),
                            )
                    for i, qb in enumerate(qbs):
                        fin(qb, g, ps[i])

                def k_fin(qb, g, psk):
                    sl = slice(QB * qb, QB * (qb + 1))
                    nc.vector.tensor_scalar_add(KT[g][:, sl], psk[:], bkT[:, g : g + 1])

                def k_proj(qb, g):
                    kq_pair(wk16t, (qb,), g, k_fin)

                def s_proj(qb):
                    # s = 1 + 0.05*tanh((z+bs)/2); tanh is in the exp table set
                    sl = slice(QB * qb, QB * (qb + 1))
                    pss = ppsum.tile([128, QB], F32, tag="psproj")
                    for j2 in range(4):
                        nc.tensor.matmul(
                            pss[0:64, :], ws8t[:, j2], h8t[:, qb, j2],
                            start=(j2 == 0), stop=(j2 == 3), perf_mode=DR,
                        )
                    th = small.tile([64, QB], F32, tag="sexp")
                    nc.scalar.activation(th[:], pss[0:64, :], AF.Tanh,
                                         scale=0.5, bias=bsH[:, 0:1])
                    nc.vector.tensor_scalar(sval[:, sl], th[:], 0.05, 1.0, ALU.mult, ALU.add)
                    for g in range(NG):
                        nc.sync.dma_start(
                            srep[g][:, sl],
                            sval[32 * g : 32 * (g + 1), sl][:, None, :]
                            .to_broadcast((32, 4, QB)),
                        )

                def q_fin(qb, g, psq):
                    sl = slice(QB * qb, QB * (qb + 1))
                    nc.vector.scalar_tensor_tensor(
                        QT[g][:, sl], psq[:], bqT[:, g : g + 1], srep[g][:, sl],
                        ALU.add, ALU.mult,
                    )

                def q_proj(qb, g):
                    kq_pair(wq16t, (qb,), g, q_fin)

                def v_proj(t):
                    psv = ppsum.tile([128, QB], F32, tag="psproj")
                    for j in range(EJ):
                        nc.tensor.matmul(
                            psv[:, 0 : HL * DH],
                            h16t[:, j, 128 * t : 128 * (t + 1)],
                            wv16t[:, j, :],
                            start=(j == 0), stop=(j == EJ - 1),
                        )
                    va3 = VA[t].rearrange("p (h x) -> p h x", h=HL)
                    nc.vector.tensor_tensor(
                        va3[:, :, 0:64],
                        psv[:, 0 : HL * DH].rearrange("p (h d) -> p h d", h=HL),
                        bv_rep.rearrange("p (h d) -> p h d", h=HL),
                        ALU.add,
                    )

                # ---- preamble ----
                k_proj(0, 0)
                k_proj(0, 1)
                s_proj(0)
                q_proj(0, 0)
                q_proj(0, 1)

                fillers = {
                    0: ([lambda g=g: kq_pair(wk16t, (1, 2), g, k_fin) for g in (0, 1)]
                        + [lambda g=g: k_proj(3, g) for g in (0, 1)]
                        + [lambda t=t: v_proj(t) for t in range(7)]),
                    1: ([lambda t=t: v_proj(t) for t in range(7, ST)]
                        + [lambda: s_proj(1), lambda: q_proj(1, 0), lambda: q_proj(1, 1)]),
                    2: [lambda: s_proj(2), lambda: s_proj(3),
                        lambda: kq_pair(wq16t, (2, 3), 0, q_fin)],
                    3: [lambda: kq_pair(wq16t, (2, 3), 1, q_fin)],
                }

                cs_bufs = [
                    ctxp.tile([128, QB], FP16, tag=f"cs{i}", name=f"{pfx}cs{i}")
                    for i in range(4)
                ]
                for c in cs_bufs:
                    nc.gpsimd.memset(c[:], 0.0)

                with (
                    tc.tile_pool(name=pfx + "epsum", bufs=2, space="PSUM") as epsum,
                    tc.tile_pool(name=pfx + "psctx", bufs=2, space="PSUM") as psctx,
                ):
                    all_ets = {}
                    psc_live = {}
                    outs_live = {}

                    def scores_step(sec, t):
                        qblk, g = divmod(sec, NG)
                        qsl = slice(QB * qblk, QB * (qblk + 1))
                        pss = epsum.tile([128, 2 * QB], F32, tag="psbig")
                        for sub in range(2):
                            hb = 64 * sub
                            nc.tensor.matmul(
                                pss[:, QB * sub : QB * (sub + 1)],
                                KT[g][hb : hb + 64, 128 * t : 128 * (t + 1)],
                                QT[g][hb : hb + 64, qsl],
                                start=True, stop=True,
                            )
                        return pss

                    def exp_step(sec, t, pss):
                        et = expp.tile([128, 2 * QB], FP16, tag="et",
                                       name=f"{pfx}et{sec}_{t}")
                        all_ets[(sec, t)] = et
                        if t in DVE_T:
                            nc.vector.tensor_scalar(
                                et.bitcast(U16)[:], pss[:],
                                SCH_A, SCH_B, ALU.mult, ALU.add,
                            )
                        else:
                            nc.scalar.activation(
                                et[:], pss[:], AF.Exp,
                                scale=SCALE, bias=bneg[:, 0:1],
                            )

                    def ctx_step(sec, t):
                        _, g = divmod(sec, NG)
                        et3 = all_ets[(sec, t)].rearrange("p (s q) -> p s q", s=2)
                        for sub in range(2):
                            head = 2 * g + sub
                            if t == 0:
                                if sec == NSEC - 1:
                                    pt = ppsum.tile(
                                        [128, QB], F32, tag="psproj",
                                        name=f"{pfx}psc{sec}_{sub}",
                                    )
                                    psc_live[(sec, sub)] = pt[0:65, :]
                                else:
                                    psc_live[(sec, sub)] = psctx.tile(
                                        [65, QB], F32, tag="psc",
                                        name=f"{pfx}psc{sec}_{sub}",
                                    )
                            nc.tensor.matmul(
                                psc_live[(sec, sub)],
                                VA[t][:, 65 * head : 65 * (head + 1)],
                                et3[:, sub],
                                start=(t == 0), stop=(t == ST - 1),
                            )
                        del all_ets[(sec, t)]

                    def ctx_finish(sec):
                        qblk, g = divmod(sec, NG)
                        if g == 0:
                            outs_live[qblk] = outp.tile(
                                [128, 4, HL * DH], F32, tag="out_sb",
                                name=f"{pfx}out_sb_{qblk}",
                            )
                        outs = outs_live[qblk]
                        for sub in range(2):
                            head = 2 * g + sub
                            psc = psc_live.pop((sec, sub))
                            cs = cs_bufs[2 * g + sub]
                            nc.vector.tensor_copy(cs[0:65, :], psc[:])
                            pst = tpd.tile([128, 4, 128], FP16, tag="pst")
                            ring = nc.sync if sub == 0 else nc.scalar
                            ring.dma_start_transpose(pst[:], cs[:])
                            rec = small.tile([128, 4], F32, tag="rec")
                            nc.vector.reciprocal(
                                rec[:],
                                pst[:, :, 64:65].rearrange("p a b -> p (a b)"),
                            )
                            nc.vector.tensor_tensor(
                                outs[:, :, DH * head : DH * (head + 1)],
                                pst[:, :, 0:64],
                                rec[:, :, None].to_broadcast((128, 4, 64)),
                                ALU.mult,
                            )
                        if g == NG - 1:
                            qsl = slice(QB * qblk, QB * (qblk + 1))
                            nc.sync.dma_start(
                                out[qsl].rearrange("(qt p) c -> p qt c", p=128),
                                outs[:],
                            )

                    for sec in range(NSEC):
                        fl = fillers.get(sec, [])
                        fi = 0
                        last = sec == NSEC - 1
                        for t in range(ST):
                            pss = scores_step(sec, t)
                            if t >= 4 and fi < len(fl):
                                fl[fi]()
                                fi += 1
                            exp_step(sec, t, pss)
                            if sec >= 1:
                                ctx_step(sec - 1, t)
                                if t == ST - 1:
                                    ctx_finish(sec - 1)
                            if last and t >= 1:
                                ctx_step(sec, t - 1)
                        while fi < len(fl):
                            fl[fi]()
                            fi += 1
                    ctx_step(NSEC - 1, ST - 1)
                    ctx_finish(NSEC - 1)

        for rep in range(reps):
            emit(f"R{rep}" if reps > 1 else "")
    return nc


_NC = None


def _get_compiled():
    global _NC
    if _NC is None:
        nc = bacc.Bacc(
            "TRN2",
            target_bir_lowering=False,
            debug=False,
            enable_asserts=False,
            num_devices=8,
        )
        build_program(nc)
        nc.compile()
        _NC = nc
    return _NC


def _ws8(a):
    # [64, E] -> [128, 4, 2, 64]: (k, j2, o, d) = a[d, 256*j2+128*o+k]
    import ml_dtypes
    at = np.asarray(a, np.float32).T.reshape(4, 2, 128, 64).transpose(2, 0, 1, 3)
    return np.ascontiguousarray(at.astype(ml_dtypes.float8_e4m3))


def _h8(a):
    # [S, E] -> [128, 4, 4, 2, 512]: (k, qb, j2, o, s') = a[512*qb+s', 256*j2+128*o+k]
    import ml_dtypes
    at = np.asarray(a, np.float32).T.reshape(4, 2, 128, 4, 512).transpose(2, 3, 0, 1, 4)
    return np.ascontiguousarray(at.astype(ml_dtypes.float8_e4m3))


def make_in_maps(hidden_states, Wq, bq, Wk, bk, Wv, bv, Ws, bs):
    def tr16(a):
        # [rows, E] -> [128, EJ, rows] fp16: (k, j, r) = a[r, 128j+k]
        at = np.asarray(a, np.float32).T.astype(np.float16)   # [E, rows]
        return np.ascontiguousarray(at.reshape(EJ, 128, -1).transpose(1, 0, 2))

    def trh(a):
        at = np.asarray(a, np.float32).T.astype(np.float16)       # [E, S]
        at = at.reshape(EJ, 128, 4, 512).transpose(1, 2, 0, 3)     # [k, qb, j, s']
        return np.ascontiguousarray(at)

    in_maps = []
    for c in range(8):
        b, hq = divmod(c, 4)
        r = slice(256 * hq, 256 * (hq + 1))
        rs = slice(64 * hq, 64 * (hq + 1))
        in_maps.append(
            {
                "h16": trh(hidden_states[b]),
                "wk16": tr16(Wk[r]), "wq16": tr16(Wq[r]),
                "ws8": _ws8(Ws[rs]), "h8": _h8(hidden_states[b]),
                "wv16": tr16(Wv[r]),
                "bqt": np.ascontiguousarray(np.asarray(bq[r], np.float32).reshape(2, 128).T),
                "bkt": np.ascontiguousarray(np.asarray(bk[r], np.float32).reshape(2, 128).T),
                "bsh": np.ascontiguousarray((np.asarray(bs[rs], np.float32) * 0.5).reshape(64, 1)),
                "bv": np.ascontiguousarray(np.asarray(bv[r], np.float32)),
            }
        )
    return in_maps


def assemble(results):
    out = np.empty((2, S, 1024), np.float32)
    for c in range(8):
        b, hq = divmod(c, 4)
        out[b, :, 256 * hq : 256 * (hq + 1)] = results[c]["out"]
    return out


def kernel(hidden_states, Wq, bq, Wk, bk, Wv, bv, Ws, bs):
    nc = _get_compiled()
    in_maps = make_in_maps(hidden_states, Wq, bq, Wk, bk, Wv, bv, Ws, bs)
    # First execution after NEFF load can race input transfer (observed
    # corrupt outputs); run once to warm, return the second run's result.
    bass_utils.run_bass_kernel_spmd(nc, in_maps, core_ids=list(range(8)))
    res = bass_utils.run_bass_kernel_spmd(nc, in_maps, core_ids=list(range(8)))
    return assemble(res.results)
